# revision 39
# baseline (speedup 1.0000x reference)
"""DGCNN (SGConv K=2 + conv-pool + fc) Trainium2 kernel.

Math:
  A_norm = D^-1/2 (A + I) D^-1/2   (A from tril edge_w, symmetrized)
  h      = relu(A_norm^2 @ x @ lin_w + lin_b)        [B, N, H]
  pooled = relu(einsum('bnh,n->bh', h, conv_w) + conv_b)
  out    = pooled @ fc_w + fc_b                      [B, C]

Device strategy (data-parallel over batch, 8 cores x 512 batches):
  Host folds the two SGConv hops into A2 = A_norm @ A_norm and folds
  |conv_w| into A2's columns. Using c*relu(z) = sign(c)*relu(|c|*z),
  the node pooling becomes a +-1-weighted cross-partition sum, which runs
  on the tensor engine as a [128,1] matmul. Host pre-transposes x to
  f-major so the f-contraction (lin) can use x as the self-loading
  stationary operand; the j-contraction (A2) and pooling then need no
  transposes at all.

  per 8-batch group (all 16-bit tensors fp16; PSUM accumulates fp32):
    MM_L x8 : z[j,h]   = x_b @ lin_w     (lhsT = xT_b slice)   -> 1 PSUM bank
    DVE     : z  PSUM -> SBUF (fp16, 2x mode)
    MM_A    : Z2[i,bh] = A2c^T @ z       (lhsT = A2*|c|, N=512)
    ACT     : U = relu(Z2)  PSUM -> SBUF (fp16)
    MM_P    : pooled[1,bh] = sign(c)^T @ U  (tile_position col-packed,
              4 groups per PSUM bank; one ACT copy + DMA per 4 groups)
  x is DMA'd in 2 MB chunks (8 groups) — small transfers run ~250 GB/s,
  2 MB runs ~376 GB/s. Stages are software-pipelined with lags so the PE
  never waits on the DVE/ACT round trips. Measured ~84 us/core; rel-err
  ~1.7e-3 vs the fp32 reference.

  Host epilogue: relu(pooled + conv_b) @ fc_w + fc_b  on [B, 64].
"""

import ml_dtypes
import numpy as np

import concourse.bacc as bacc
import concourse.bass as bass
import concourse.bass_isa as bass_isa
import concourse.mybir as mybir
import concourse.tile as tile
from concourse.bass_utils import run_bass_kernel_spmd

N = 128       # nodes
F_IN = 128    # in features
H = 64        # hidden
C = 40        # classes
B = 4096      # batch
NCORES = 8
BPC = B // NCORES          # 512 batches per core
G = 8                      # batches per group (fills one PSUM bank: 8*64=512 fp32)
NG = BPC // G              # 64 groups
CHUNK = 16                 # groups per x DMA (4 MB transfers)
NCHUNK = NG // CHUNK

F32 = mybir.dt.float32
F32R = mybir.dt.float32r
BF16 = mybir.dt.bfloat16
FP16 = mybir.dt.float16
RELU = mybir.ActivationFunctionType.Relu

# fp16 everywhere for the 2-byte tensors: halves DMA traffic, 1 cyc/row
# on the PE with FWL weight loads, 2x DVE/ACT modes, and its 10-bit
# mantissa keeps the end-to-end error at ~1.7e-3 (vs 1.2e-2 for bf16).
MM_DT = FP16
X_DT = FP16
PFLUSH = 4  # pooled rows per PSUM bank / per ACT copy
LAGA = 6    # pipeline lag of the A2-hop stage behind the lin stage
LAGP = 12   # pipeline lag of the pooling stage
PREFETCH = 2  # x chunks DMA'd ahead of first use (must be < xin bufs)

# Pool-engine pooling via gpsimd partition_all_reduce: measured ~18 us
# per [128, 4096] block (~0.25 GB/s) — 3.4x SLOWER overall. Keep off;
# pooling stays on the PE as a [1,512] matmul per group.
POOL_ENGINE = False
POOLB = 8   # groups per gpsimd flush
LAGF = 6    # pipeline lag of the gpsimd flush behind the lin stage

_PROG_CACHE: dict = {}
_last_in_maps: list = []
_LAST_NPOS: int = 64
NBLK = NG // POOLB

# ablation knob for benchmarking: 'full', 'no_p' (skip pooling MM+copy+out),
# 'lin_only' (skip A2 hop too), 'dma_only' (skip all compute)
_VARIANT = "full"


def _build_program(has_bias: bool, repeat: int = 1, mult: int = 1,
                   npos: int | None = None):
    """repeat > 1 wraps the whole workload in a device-side For_i loop;
    mult replicates the workload inside the loop body. Benchmarking only:
    the (mult=2) - (mult=1) slope at fixed repeat cancels both dispatch
    and the For_i back-edge overhead exactly."""
    pooleng = POOL_ENGINE and _VARIANT == "full"
    nc = bacc.Bacc(
        "TRN2", target_bir_lowering=False, debug=False, num_devices=NCORES
    )
    # Flat last dim: the chunk DMA lowers to a plain 2D [128, CHUNK*G*N]
    # copy (contiguous per partition). The 3D `p b j` form produced
    # descriptor-fragmented DMAs (~100 GB/s vs ~400 GB/s).
    xP = nc.declare_dram_parameter(
        "xP", [NCHUNK, F_IN, CHUNK * G * N], X_DT, isOutput=False
    )
    a2c = nc.declare_dram_parameter("a2c", [N, N], MM_DT, isOutput=False)
    linw = nc.declare_dram_parameter("linw", [F_IN, H], X_DT, isOutput=False)
    scol = nc.declare_dram_parameter(
        "scol", [N, 1], F32 if pooleng else MM_DT, isOutput=False
    )
    if has_bias:
        btile = nc.declare_dram_parameter("btile", [N, G * H], F32, isOutput=False)
    if pooleng:
        pooled_blk = nc.declare_dram_parameter(
            "pooled_blk", [NBLK, POOLB * G * H], F32, isOutput=True
        )
    else:
        pooled = nc.declare_dram_parameter("pooled", [BPC, H], F32, isOutput=True)

    with tile.TileContext(nc) as tc:
        with (
            tc.tile_pool(name="const", bufs=1) as constp,
            tc.tile_pool(name="xin", bufs=3) as xinp,
            tc.tile_pool(name="zs", bufs=9) as zsp,
            tc.tile_pool(name="u", bufs=(2 if pooleng else 10)) as up,
            tc.tile_pool(name="pr", bufs=2) as prp,
            tc.tile_pool(name="ob", bufs=3) as obp,
            tc.tile_pool(name="psL", bufs=3, space="PSUM") as psL,
            tc.tile_pool(name="psA", bufs=3, space="PSUM") as psA,
            tc.tile_pool(name="psP", bufs=2, space="PSUM") as psP,
        ):
            a2c_t = constp.tile([N, N], MM_DT)
            nc.sync.dma_start(a2c_t[:], a2c[:, :])
            linw_t = constp.tile([F_IN, H], X_DT)
            nc.sync.dma_start(linw_t[:], linw[:, :])
            scol_t = constp.tile([N, 1], F32 if pooleng else MM_DT)
            nc.sync.dma_start(scol_t[:], scol[:, :])
            if has_bias:
                bt_t = constp.tile([N, G * H], F32)
                nc.sync.dma_start(bt_t[:], btile[:, :])

            import contextlib

            # Software pipeline with lag so the PE never stalls on the
            # per-group PE->DVE->PE->ACT->PE round trips: stage A (the A2
            # hop) runs one group behind stage L, pooling two groups behind.
            zst_q: dict = {}
            ut_q: dict = {}
            zps_q: dict = {}

            X_tiles: dict = {}

            def issue_x_dma(c):
                # Prefetch: traced PREFETCH chunks before first use, so the
                # ~5 us transfer is in flight long before MM_L needs it.
                if c < NCHUNK and c not in X_tiles:
                    X8 = xinp.tile([F_IN, CHUNK * G * N], X_DT, name="X8", tag="X")
                    nc.sync.dma_start(X8[:], xP[c])
                    X_tiles[c] = X8

            def stage_L_first(i):
                if i % CHUNK == 0:
                    issue_x_dma(i // CHUNK + PREFETCH)
                X = X_tiles[i // CHUNK]
                off = (i % CHUNK) * G * N
                zps = psL.tile([N, G * H], F32, tag="zps")
                zps_q[i] = zps
                for b in range(G // 2):
                    nc.tensor.matmul(
                        zps[:, b * H : (b + 1) * H],
                        lhsT=X[:, off + b * N : off + (b + 1) * N],
                        rhs=linw_t[:],
                        start=True,
                        stop=True,
                    )

            def stage_L_second(i):
                X = X_tiles[i // CHUNK]
                off = (i % CHUNK) * G * N
                zps = zps_q.pop(i)
                for b in range(G // 2, G):
                    nc.tensor.matmul(
                        zps[:, b * H : (b + 1) * H],
                        lhsT=X[:, off + b * N : off + (b + 1) * N],
                        rhs=linw_t[:],
                        start=True,
                        stop=True,
                    )
                zst = zsp.tile([N, G * H], MM_DT, tag="zst")
                # The two per-group PSUM->SBUF ops (this copy + the relu in
                # stage_A) are each ~533 ns at 1 elem/lane/cycle — one engine
                # doing both would pace the whole kernel. Alternate them
                # across DVE and ACT so each engine carries one per group.
                if i % 2 == 0:
                    nc.vector.tensor_copy(zst[:], zps[:])
                else:
                    nc.scalar.copy(zst[:], zps[:])
                zst_q[i] = zst

            ustage_q: dict = {}

            def stage_A(i):
                zst = zst_q.pop(i)
                z2 = psA.tile([N, G * H], F32, tag="z2")
                nc.tensor.matmul(
                    z2[:], lhsT=a2c_t[:], rhs=zst[:], start=True, stop=True
                )
                if has_bias:
                    zin = zsp.tile([N, G * H], F32, tag="zb")
                    nc.vector.tensor_add(zin[:], z2[:], bt_t[:])
                else:
                    zin = z2
                if pooleng:
                    t = i // POOLB
                    if i % POOLB == 0:
                        ustage_q[t] = up.tile(
                            [N, POOLB * G * H], MM_DT, name="ustage",
                            tag="ustage",
                        )
                    k = i % POOLB
                    nc.scalar.activation(
                        ustage_q[t][:, k * G * H : (k + 1) * G * H],
                        zin[:],
                        RELU,
                    )
                else:
                    ut = up.tile([N, G * H], MM_DT, tag="ut")
                    if i % 2 == 0:
                        nc.scalar.activation(ut[:], zin[:], RELU)
                    else:
                        nc.vector.tensor_scalar(
                            ut[:], zin[:], 0.0, None, mybir.AluOpType.max
                        )
                    ut_q[i] = ut

            def flush_pool(t):
                # Weighted node pooling off the PE: apply sign(c) as an
                # in-place per-partition multiply (alternating DVE/ACT so
                # neither engine saturates), then one 128-partition add
                # all-reduce on the otherwise-idle GpSimd engine.
                ust = ustage_q.pop(t)
                if t % 2 == 0:
                    nc.vector.tensor_scalar(
                        ust[:], ust[:], scol_t[:], None,
                        mybir.AluOpType.mult,
                    )
                else:
                    nc.scalar.mul(ust[:], ust[:], scol_t[:])
                pr = prp.tile([N, POOLB * G * H], F32, tag="prp")
                nc.gpsimd.partition_all_reduce(
                    pr[:], ust[:], channels=N,
                    reduce_op=bass_isa.ReduceOp.add,
                )
                nc.scalar.dma_start(pooled_blk[t].unsqueeze(0), pr[0:1, :])

            pps_cur: list = [None]

            def stage_P(i):
                # MM_P(i) writes one partition row (32*(i%PFLUSH)) of a shared
                # PSUM bank; every PFLUSH groups one ACT copy + one DMA flush
                # the whole bank. Cuts the [1,512]-row copy cost by PFLUSH.
                k = i % PFLUSH
                if k == 0:
                    pps_cur[0] = psP.tile([128, G * H], F32, name="pps", tag="pps")
                pps = pps_cur[0]
                ut = ut_q.pop(i)
                nc.tensor.matmul(
                    pps[32 * k : 32 * k + 1, :],
                    lhsT=scol_t[:],
                    rhs=ut[:],
                    start=True,
                    stop=True,
                    tile_position=(0, 32 * k),
                )
                if k == PFLUSH - 1:
                    # Copy cost is free-dim-based, so the whole 97-partition
                    # span costs the same as one row; the DMA then strides
                    # over partitions (PSUM can't be a DMA source). Alternate
                    # the copy engine; DMA goes on the ACT HWDGE ring so the
                    # x-chunk DMAs on the SP ring are not delayed.
                    i0 = i - (PFLUSH - 1)
                    nrows = 32 * (PFLUSH - 1) + 1
                    ob = obp.tile([nrows, G * H], F32, tag="ob")
                    if (i // PFLUSH) % 2 == 0:
                        nc.vector.tensor_copy(ob[:], pps[0:nrows, :])
                    else:
                        nc.scalar.copy(ob[:], pps[0:nrows, :])
                    nc.scalar.dma_start(
                        pooled[i0 * G : (i + 1) * G].rearrange(
                            "(a b) h -> a (b h)", a=PFLUSH
                        ),
                        ob[0:nrows:32, :],
                    )

            loop_cm = (
                tc.For_i(0, repeat, 1) if repeat > 1 else contextlib.nullcontext()
            )
            def stage_L_dma_only(c):
                X8 = xinp.tile([F_IN, CHUNK * G * N], X_DT, name="X8d", tag="X")
                nc.sync.dma_start(X8[:], xP[c])
                if c == NCHUNK - 1:  # write something so output is bound
                    ob = obp.tile([1, G * H], F32, tag="ob")
                    nc.vector.tensor_copy(ob[:], X8[0:1, 0 : 2 * G * H].bitcast(F32))
                    nc.sync.dma_start(
                        pooled[0:G].rearrange("b h -> (b h)").unsqueeze(0),
                        ob[:],
                    )

            with loop_cm:
                for _ in range(mult):
                    if _VARIANT == "dma_only":
                        for c in range(NCHUNK):
                            stage_L_dma_only(c)
                        continue
                    for c in range(PREFETCH):
                        issue_x_dma(c)
                    # Per i, PE order: MM_A (big stream), 4x MM_L, MM_P (big
                    # stream), 4x MM_L. The big streams give the LDWEIGHTS
                    # pull-ahead window room to hide the next x loads.
                    tail = LAGF if pooleng else LAGP
                    for i in range(NG + tail):
                        if LAGA <= i < NG + LAGA and _VARIANT in ("full", "no_p"):
                            stage_A(i - LAGA)
                        if i < NG:
                            stage_L_first(i)
                        if not pooleng and i >= LAGP and _VARIANT == "full":
                            stage_P(i - LAGP)
                        if (pooleng and i >= LAGF
                                and (i - LAGF) % POOLB == POOLB - 1):
                            flush_pool((i - LAGF) // POOLB)
                        if i < NG:
                            stage_L_second(i)
                    X_tiles.clear()
                    if _VARIANT in ("no_p", "lin_only"):
                        # bind the output with a dummy write
                        src = zst_q[NG - 1] if _VARIANT == "lin_only" else ut_q[NG - 1]
                        ob = obp.tile([1, 256], F32, tag="obd")
                        nc.vector.tensor_copy(ob[:], src[0:1, :].bitcast(F32))
                        nc.sync.dma_start(
                            pooled[0:4].rearrange("b h -> (b h)").unsqueeze(0),
                            ob[:],
                        )
                        zst_q.clear()
                        ut_q.clear()
    nc.compile()
    return nc


def _get_program(has_bias: bool):
    key = (has_bias, MM_DT, POOL_ENGINE)
    if key not in _PROG_CACHE:
        _PROG_CACHE[key] = _build_program(has_bias)
    return _PROG_CACHE[key]


def _host_adjacency(edge_w, conv_w):
    """A2*|c| (lhsT layout) and sign(c) column, in float64 then cast."""
    ew = np.asarray(edge_w, dtype=np.float64)
    A = np.zeros((N, N), dtype=np.float64)
    xs, ys = np.tril_indices(N)
    A[xs, ys] = ew
    A = A + A.T - np.diag(np.diag(A))
    Ah = A + np.eye(N)
    deg = Ah.sum(axis=1)
    dinv = np.where(deg > 0, deg ** -0.5, 0.0)
    An = dinv[:, None] * Ah * dinv[None, :]
    A2 = An @ An
    c = np.asarray(conv_w, dtype=np.float64)
    # out[i,x] = sum_j lhsT[j,i] rhs[j,x]; want sum_j A2[i,j]|c_i| z[j,x]
    # lhsT[j,i] = A2[i,j]*|c_i| = A2[j,i]*|c_i| (A2 symmetric)
    a2c = (A2 * np.abs(c)[None, :]).astype(np.float32)
    scol = np.sign(c).astype(np.float32).reshape(N, 1)
    return np.ascontiguousarray(a2c), scol


def _run(inputs: dict, trace: bool = False):
    x = np.asarray(inputs["x"], dtype=np.float32)
    edge_w = np.asarray(inputs["edge_w"], dtype=np.float32)
    lin_w = np.ascontiguousarray(np.asarray(inputs["lin_w"], dtype=np.float32))
    lin_b = np.asarray(inputs["lin_b"], dtype=np.float32)
    conv_w = np.asarray(inputs["conv_w"], dtype=np.float32)
    conv_b = np.asarray(inputs["conv_b"], dtype=np.float32)
    fc_w = np.asarray(inputs["fc_w"], dtype=np.float32)
    fc_b = np.asarray(inputs["fc_b"], dtype=np.float32)

    a2c, scol = _host_adjacency(edge_w, conv_w)
    has_bias = bool(np.any(lin_b != 0))
    nc = _get_program(has_bias)

    _np_of = {F32: np.float32, BF16: ml_dtypes.bfloat16, FP16: np.float16}
    np_xdt = _np_of[X_DT]
    np_mmdt = _np_of[MM_DT]
    linw_dev = lin_w.astype(np_xdt)
    a2c = a2c.astype(np_mmdt)
    scol = scol.astype(np.float32 if POOL_ENGINE else np_mmdt)
    in_maps = []
    for k in range(NCORES):
        xc = x[k * BPC : (k + 1) * BPC]                  # [512, j, f]
        xc = xc.reshape(NCHUNK, CHUNK * G, N, F_IN)      # [c, b, j, f]
        xPk = np.ascontiguousarray(
            xc.transpose(0, 3, 1, 2).astype(np_xdt)
        ).reshape(NCHUNK, F_IN, CHUNK * G * N)  # [c, f, b*j]
        m = {"xP": xPk, "a2c": a2c, "linw": linw_dev, "scol": scol}
        if has_bias:
            bt = np.abs(conv_w.astype(np.float64))[:, None] * lin_b.astype(np.float64)[None, :]
            m["btile"] = np.ascontiguousarray(
                np.tile(bt.astype(np.float32), (1, G))
            )
        in_maps.append(m)

    global _last_in_maps
    _last_in_maps = in_maps
    try:
        res = run_bass_kernel_spmd(nc, in_maps, list(range(NCORES)), trace=trace)
    except ModuleNotFoundError:
        # no NTFF profiling hook in this environment
        res = run_bass_kernel_spmd(nc, in_maps, list(range(NCORES)), trace=False)
    if POOL_ENGINE:
        pooled = np.concatenate(
            [res.results[k]["pooled_blk"].reshape(BPC, H) for k in range(NCORES)],
            axis=0,
        )  # [B, H]
    else:
        pooled = np.concatenate(
            [res.results[k]["pooled"] for k in range(NCORES)], axis=0
        )  # [B, H]

    p = np.maximum(pooled + conv_b[0], 0.0).astype(np.float32)
    out = (p @ fc_w + fc_b).astype(np.float32)
    return out, res


def kernel(x, edge_w, lin_w, lin_b, conv_w, conv_b, fc_w, fc_b):
    out, _ = _run(
        {
            "x": x,
            "edge_w": edge_w,
            "lin_w": lin_w,
            "lin_b": lin_b,
            "conv_w": conv_w,
            "conv_b": conv_b,
            "fc_w": fc_w,
            "fc_b": fc_b,
        }
    )
    return out



# revision 40
# speedup vs baseline: 1.3503x; 1.3503x over previous
"""DGCNN (SGConv K=2 + conv-pool + fc) Trainium2 kernel.

Math:
  A_norm = D^-1/2 (A + I) D^-1/2   (A from tril edge_w, symmetrized)
  h      = relu(A_norm^2 @ x @ lin_w + lin_b)        [B, N, H]
  pooled = relu(einsum('bnh,n->bh', h, conv_w) + conv_b)
  out    = pooled @ fc_w + fc_b                      [B, C]

Device strategy (data-parallel over batch, 8 cores x 512 batches):
  Host folds the two SGConv hops into A2 = A_norm @ A_norm and folds
  |conv_w| into A2's columns. Using c*relu(z) = sign(c)*relu(|c|*z),
  the node pooling becomes a +-1-weighted cross-partition sum, which runs
  on the tensor engine as a [128,1] matmul. Host pre-transposes x to
  f-major so the f-contraction (lin) can use x as the self-loading
  stationary operand; the j-contraction (A2) and pooling then need no
  transposes at all.

  per 8-batch group (all 16-bit tensors fp16; PSUM accumulates fp32):
    MM_L x8 : z[j,h]   = x_b @ lin_w     (lhsT = xT_b slice)   -> 1 PSUM bank
    DVE     : z  PSUM -> SBUF (fp16, 2x mode)
    MM_A    : Z2[i,bh] = A2c^T @ z       (lhsT = A2*|c|, N=512)
    ACT     : U = relu(Z2)  PSUM -> SBUF (fp16)
    MM_P    : pooled[1,bh] = sign(c)^T @ U  (tile_position col-packed,
              4 groups per PSUM bank; one ACT copy + DMA per 4 groups)
  x is DMA'd in 2 MB chunks (8 groups) — small transfers run ~250 GB/s,
  2 MB runs ~376 GB/s. Stages are software-pipelined with lags so the PE
  never waits on the DVE/ACT round trips. Measured ~84 us/core; rel-err
  ~1.7e-3 vs the fp32 reference.

  Host epilogue: relu(pooled + conv_b) @ fc_w + fc_b  on [B, 64].
"""

import ml_dtypes
import numpy as np

import concourse.bacc as bacc
import concourse.bass as bass
import concourse.bass_isa as bass_isa
import concourse.mybir as mybir
import concourse.tile as tile
from concourse.bass_utils import run_bass_kernel_spmd

N = 128       # nodes
F_IN = 128    # in features
H = 64        # hidden
C = 40        # classes
B = 4096      # batch
NCORES = 8
BPC = B // NCORES          # 512 batches per core
G = 8                      # batches per group (fills one PSUM bank: 8*64=512 fp32)
NG = BPC // G              # 64 groups
CHUNK = 8                  # groups per x DMA (2 MB transfers: measured
                           # fastest; 4 MB chunks regress to ~66 us)
NCHUNK = NG // CHUNK

F32 = mybir.dt.float32
F32R = mybir.dt.float32r
BF16 = mybir.dt.bfloat16
FP16 = mybir.dt.float16
RELU = mybir.ActivationFunctionType.Relu

# fp16 everywhere for the 2-byte tensors: halves DMA traffic, 1 cyc/row
# on the PE with FWL weight loads, 2x DVE/ACT modes, and its 10-bit
# mantissa keeps the end-to-end error at ~1.7e-3 (vs 1.2e-2 for bf16).
MM_DT = FP16
X_DT = FP16
PFLUSH = 4  # pooled rows per PSUM bank / per ACT copy
LAGA = 6    # pipeline lag of the A2-hop stage behind the lin stage
LAGP = 12   # pipeline lag of the pooling stage
PREFETCH = 2  # x chunks DMA'd ahead of first use (must be < xin bufs)

# Pool-engine pooling via gpsimd partition_all_reduce: measured ~18 us
# per [128, 4096] block (~0.25 GB/s) — 3.4x SLOWER overall. Keep off;
# pooling stays on the PE as a [1,512] matmul per group.
POOL_ENGINE = False
POOLB = 8   # groups per gpsimd flush
LAGF = 6    # pipeline lag of the gpsimd flush behind the lin stage

_PROG_CACHE: dict = {}
_last_in_maps: list = []
_LAST_NPOS: int = 64
NBLK = NG // POOLB

# ablation knob for benchmarking: 'full', 'no_p' (skip pooling MM+copy+out),
# 'lin_only' (skip A2 hop too), 'dma_only' (skip all compute)
_VARIANT = "full"


def _build_program(has_bias: bool, repeat: int = 1, mult: int = 1,
                   npos: int | None = None):
    """repeat > 1 wraps the whole workload in a device-side For_i loop;
    mult replicates the workload inside the loop body. Benchmarking only:
    the (mult=2) - (mult=1) slope at fixed repeat cancels both dispatch
    and the For_i back-edge overhead exactly."""
    pooleng = POOL_ENGINE and _VARIANT == "full"
    nc = bacc.Bacc(
        "TRN2", target_bir_lowering=False, debug=False, num_devices=NCORES
    )
    # Flat last dim: the chunk DMA lowers to a plain 2D [128, CHUNK*G*N]
    # copy (contiguous per partition). The 3D `p b j` form produced
    # descriptor-fragmented DMAs (~100 GB/s vs ~400 GB/s).
    xP = nc.declare_dram_parameter(
        "xP", [NCHUNK, F_IN, CHUNK * G * N], X_DT, isOutput=False
    )
    a2c = nc.declare_dram_parameter("a2c", [N, N], MM_DT, isOutput=False)
    linw = nc.declare_dram_parameter("linw", [F_IN, H], X_DT, isOutput=False)
    scol = nc.declare_dram_parameter(
        "scol", [N, 1], F32 if pooleng else MM_DT, isOutput=False
    )
    if has_bias:
        btile = nc.declare_dram_parameter("btile", [N, G * H], F32, isOutput=False)
    if pooleng:
        pooled_blk = nc.declare_dram_parameter(
            "pooled_blk", [NBLK, POOLB * G * H], F32, isOutput=True
        )
    else:
        pooled = nc.declare_dram_parameter("pooled", [BPC, H], F32, isOutput=True)

    with tile.TileContext(nc) as tc:
        with (
            tc.tile_pool(name="const", bufs=1) as constp,
            tc.tile_pool(name="xin", bufs=3) as xinp,
            tc.tile_pool(name="zs", bufs=9) as zsp,
            tc.tile_pool(name="u", bufs=(2 if pooleng else 10)) as up,
            tc.tile_pool(name="pr", bufs=2) as prp,
            tc.tile_pool(name="ob", bufs=3) as obp,
            tc.tile_pool(name="psL", bufs=3, space="PSUM") as psL,
            tc.tile_pool(name="psA", bufs=3, space="PSUM") as psA,
            tc.tile_pool(name="psP", bufs=2, space="PSUM") as psP,
        ):
            a2c_t = constp.tile([N, N], MM_DT)
            nc.sync.dma_start(a2c_t[:], a2c[:, :])
            linw_t = constp.tile([F_IN, H], X_DT)
            nc.sync.dma_start(linw_t[:], linw[:, :])
            scol_t = constp.tile([N, 1], F32 if pooleng else MM_DT)
            nc.sync.dma_start(scol_t[:], scol[:, :])
            if has_bias:
                bt_t = constp.tile([N, G * H], F32)
                nc.sync.dma_start(bt_t[:], btile[:, :])

            import contextlib

            # Software pipeline with lag so the PE never stalls on the
            # per-group PE->DVE->PE->ACT->PE round trips: stage A (the A2
            # hop) runs one group behind stage L, pooling two groups behind.
            zst_q: dict = {}
            ut_q: dict = {}
            zps_q: dict = {}

            X_tiles: dict = {}

            def issue_x_dma(c):
                # Prefetch: traced PREFETCH chunks before first use, so the
                # ~5 us transfer is in flight long before MM_L needs it.
                if c < NCHUNK and c not in X_tiles:
                    X8 = xinp.tile([F_IN, CHUNK * G * N], X_DT, name="X8", tag="X")
                    nc.sync.dma_start(X8[:], xP[c])
                    X_tiles[c] = X8

            def stage_L_first(i):
                if i % CHUNK == 0:
                    issue_x_dma(i // CHUNK + PREFETCH)
                X = X_tiles[i // CHUNK]
                off = (i % CHUNK) * G * N
                zps = psL.tile([N, G * H], F32, tag="zps")
                zps_q[i] = zps
                for b in range(G // 2):
                    nc.tensor.matmul(
                        zps[:, b * H : (b + 1) * H],
                        lhsT=X[:, off + b * N : off + (b + 1) * N],
                        rhs=linw_t[:],
                        start=True,
                        stop=True,
                    )

            def stage_L_second(i):
                X = X_tiles[i // CHUNK]
                off = (i % CHUNK) * G * N
                zps = zps_q.pop(i)
                for b in range(G // 2, G):
                    nc.tensor.matmul(
                        zps[:, b * H : (b + 1) * H],
                        lhsT=X[:, off + b * N : off + (b + 1) * N],
                        rhs=linw_t[:],
                        start=True,
                        stop=True,
                    )
                zst = zsp.tile([N, G * H], MM_DT, tag="zst")
                # The two per-group PSUM->SBUF ops (this copy + the relu in
                # stage_A) are each ~533 ns at 1 elem/lane/cycle — one engine
                # doing both would pace the whole kernel. Alternate them
                # across DVE and ACT so each engine carries one per group.
                if i % 2 == 0:
                    nc.vector.tensor_copy(zst[:], zps[:])
                else:
                    nc.scalar.copy(zst[:], zps[:])
                zst_q[i] = zst

            ustage_q: dict = {}

            def stage_A(i):
                zst = zst_q.pop(i)
                z2 = psA.tile([N, G * H], F32, tag="z2")
                nc.tensor.matmul(
                    z2[:], lhsT=a2c_t[:], rhs=zst[:], start=True, stop=True
                )
                if has_bias:
                    zin = zsp.tile([N, G * H], F32, tag="zb")
                    nc.vector.tensor_add(zin[:], z2[:], bt_t[:])
                else:
                    zin = z2
                if pooleng:
                    t = i // POOLB
                    if i % POOLB == 0:
                        ustage_q[t] = up.tile(
                            [N, POOLB * G * H], MM_DT, name="ustage",
                            tag="ustage",
                        )
                    k = i % POOLB
                    nc.scalar.activation(
                        ustage_q[t][:, k * G * H : (k + 1) * G * H],
                        zin[:],
                        RELU,
                    )
                else:
                    ut = up.tile([N, G * H], MM_DT, tag="ut")
                    if i % 2 == 0:
                        nc.scalar.activation(ut[:], zin[:], RELU)
                    else:
                        nc.vector.tensor_scalar(
                            ut[:], zin[:], 0.0, None, mybir.AluOpType.max
                        )
                    ut_q[i] = ut

            def flush_pool(t):
                # Weighted node pooling off the PE: apply sign(c) as an
                # in-place per-partition multiply (alternating DVE/ACT so
                # neither engine saturates), then one 128-partition add
                # all-reduce on the otherwise-idle GpSimd engine.
                ust = ustage_q.pop(t)
                if t % 2 == 0:
                    nc.vector.tensor_scalar(
                        ust[:], ust[:], scol_t[:], None,
                        mybir.AluOpType.mult,
                    )
                else:
                    nc.scalar.mul(ust[:], ust[:], scol_t[:])
                pr = prp.tile([N, POOLB * G * H], F32, tag="prp")
                nc.gpsimd.partition_all_reduce(
                    pr[:], ust[:], channels=N,
                    reduce_op=bass_isa.ReduceOp.add,
                )
                nc.scalar.dma_start(pooled_blk[t].unsqueeze(0), pr[0:1, :])

            pps_cur: list = [None]

            def stage_P(i):
                # MM_P(i) writes one partition row (32*(i%PFLUSH)) of a shared
                # PSUM bank; every PFLUSH groups one ACT copy + one DMA flush
                # the whole bank. Cuts the [1,512]-row copy cost by PFLUSH.
                k = i % PFLUSH
                if k == 0:
                    pps_cur[0] = psP.tile([128, G * H], F32, name="pps", tag="pps")
                pps = pps_cur[0]
                ut = ut_q.pop(i)
                nc.tensor.matmul(
                    pps[32 * k : 32 * k + 1, :],
                    lhsT=scol_t[:],
                    rhs=ut[:],
                    start=True,
                    stop=True,
                    tile_position=(0, 32 * k),
                )
                if k == PFLUSH - 1:
                    # Copy cost is free-dim-based, so the whole 97-partition
                    # span costs the same as one row; the DMA then strides
                    # over partitions (PSUM can't be a DMA source). Alternate
                    # the copy engine; DMA goes on the ACT HWDGE ring so the
                    # x-chunk DMAs on the SP ring are not delayed.
                    i0 = i - (PFLUSH - 1)
                    nrows = 32 * (PFLUSH - 1) + 1
                    ob = obp.tile([nrows, G * H], F32, tag="ob")
                    if (i // PFLUSH) % 2 == 0:
                        nc.vector.tensor_copy(ob[:], pps[0:nrows, :])
                    else:
                        nc.scalar.copy(ob[:], pps[0:nrows, :])
                    nc.scalar.dma_start(
                        pooled[i0 * G : (i + 1) * G].rearrange(
                            "(a b) h -> a (b h)", a=PFLUSH
                        ),
                        ob[0:nrows:32, :],
                    )

            loop_cm = (
                tc.For_i(0, repeat, 1) if repeat > 1 else contextlib.nullcontext()
            )
            def stage_L_dma_only(c):
                X8 = xinp.tile([F_IN, CHUNK * G * N], X_DT, name="X8d", tag="X")
                nc.sync.dma_start(X8[:], xP[c])
                if c == NCHUNK - 1:  # write something so output is bound
                    ob = obp.tile([1, G * H], F32, tag="ob")
                    nc.vector.tensor_copy(ob[:], X8[0:1, 0 : 2 * G * H].bitcast(F32))
                    nc.sync.dma_start(
                        pooled[0:G].rearrange("b h -> (b h)").unsqueeze(0),
                        ob[:],
                    )

            with loop_cm:
                for _ in range(mult):
                    if _VARIANT == "dma_only":
                        for c in range(NCHUNK):
                            stage_L_dma_only(c)
                        continue
                    for c in range(PREFETCH):
                        issue_x_dma(c)
                    # Per i, PE order: MM_A (big stream), 4x MM_L, MM_P (big
                    # stream), 4x MM_L. The big streams give the LDWEIGHTS
                    # pull-ahead window room to hide the next x loads.
                    tail = LAGF if pooleng else LAGP
                    for i in range(NG + tail):
                        if LAGA <= i < NG + LAGA and _VARIANT in ("full", "no_p"):
                            stage_A(i - LAGA)
                        if i < NG:
                            stage_L_first(i)
                        if not pooleng and i >= LAGP and _VARIANT == "full":
                            stage_P(i - LAGP)
                        if (pooleng and i >= LAGF
                                and (i - LAGF) % POOLB == POOLB - 1):
                            flush_pool((i - LAGF) // POOLB)
                        if i < NG:
                            stage_L_second(i)
                    X_tiles.clear()
                    if _VARIANT in ("no_p", "lin_only"):
                        # bind the output with a dummy write
                        src = zst_q[NG - 1] if _VARIANT == "lin_only" else ut_q[NG - 1]
                        ob = obp.tile([1, 256], F32, tag="obd")
                        nc.vector.tensor_copy(ob[:], src[0:1, :].bitcast(F32))
                        nc.sync.dma_start(
                            pooled[0:4].rearrange("b h -> (b h)").unsqueeze(0),
                            ob[:],
                        )
                        zst_q.clear()
                        ut_q.clear()
    nc.compile()
    return nc


def _get_program(has_bias: bool):
    key = (has_bias, MM_DT, POOL_ENGINE)
    if key not in _PROG_CACHE:
        _PROG_CACHE[key] = _build_program(has_bias)
    return _PROG_CACHE[key]


def _host_adjacency(edge_w, conv_w):
    """A2*|c| (lhsT layout) and sign(c) column, in float64 then cast."""
    ew = np.asarray(edge_w, dtype=np.float64)
    A = np.zeros((N, N), dtype=np.float64)
    xs, ys = np.tril_indices(N)
    A[xs, ys] = ew
    A = A + A.T - np.diag(np.diag(A))
    Ah = A + np.eye(N)
    deg = Ah.sum(axis=1)
    dinv = np.where(deg > 0, deg ** -0.5, 0.0)
    An = dinv[:, None] * Ah * dinv[None, :]
    A2 = An @ An
    c = np.asarray(conv_w, dtype=np.float64)
    # out[i,x] = sum_j lhsT[j,i] rhs[j,x]; want sum_j A2[i,j]|c_i| z[j,x]
    # lhsT[j,i] = A2[i,j]*|c_i| = A2[j,i]*|c_i| (A2 symmetric)
    a2c = (A2 * np.abs(c)[None, :]).astype(np.float32)
    scol = np.sign(c).astype(np.float32).reshape(N, 1)
    return np.ascontiguousarray(a2c), scol


def _run(inputs: dict, trace: bool = False):
    x = np.asarray(inputs["x"], dtype=np.float32)
    edge_w = np.asarray(inputs["edge_w"], dtype=np.float32)
    lin_w = np.ascontiguousarray(np.asarray(inputs["lin_w"], dtype=np.float32))
    lin_b = np.asarray(inputs["lin_b"], dtype=np.float32)
    conv_w = np.asarray(inputs["conv_w"], dtype=np.float32)
    conv_b = np.asarray(inputs["conv_b"], dtype=np.float32)
    fc_w = np.asarray(inputs["fc_w"], dtype=np.float32)
    fc_b = np.asarray(inputs["fc_b"], dtype=np.float32)

    a2c, scol = _host_adjacency(edge_w, conv_w)
    has_bias = bool(np.any(lin_b != 0))
    nc = _get_program(has_bias)

    _np_of = {F32: np.float32, BF16: ml_dtypes.bfloat16, FP16: np.float16}
    np_xdt = _np_of[X_DT]
    np_mmdt = _np_of[MM_DT]
    linw_dev = lin_w.astype(np_xdt)
    a2c = a2c.astype(np_mmdt)
    scol = scol.astype(np.float32 if POOL_ENGINE else np_mmdt)
    in_maps = []
    for k in range(NCORES):
        xc = x[k * BPC : (k + 1) * BPC]                  # [512, j, f]
        xc = xc.reshape(NCHUNK, CHUNK * G, N, F_IN)      # [c, b, j, f]
        xPk = np.ascontiguousarray(
            xc.transpose(0, 3, 1, 2).astype(np_xdt)
        ).reshape(NCHUNK, F_IN, CHUNK * G * N)  # [c, f, b*j]
        m = {"xP": xPk, "a2c": a2c, "linw": linw_dev, "scol": scol}
        if has_bias:
            bt = np.abs(conv_w.astype(np.float64))[:, None] * lin_b.astype(np.float64)[None, :]
            m["btile"] = np.ascontiguousarray(
                np.tile(bt.astype(np.float32), (1, G))
            )
        in_maps.append(m)

    global _last_in_maps
    _last_in_maps = in_maps
    try:
        res = run_bass_kernel_spmd(nc, in_maps, list(range(NCORES)), trace=trace)
    except ModuleNotFoundError:
        # no NTFF profiling hook in this environment
        res = run_bass_kernel_spmd(nc, in_maps, list(range(NCORES)), trace=False)
    if POOL_ENGINE:
        pooled = np.concatenate(
            [res.results[k]["pooled_blk"].reshape(BPC, H) for k in range(NCORES)],
            axis=0,
        )  # [B, H]
    else:
        pooled = np.concatenate(
            [res.results[k]["pooled"] for k in range(NCORES)], axis=0
        )  # [B, H]

    p = np.maximum(pooled + conv_b[0], 0.0).astype(np.float32)
    out = (p @ fc_w + fc_b).astype(np.float32)
    return out, res


def kernel(x, edge_w, lin_w, lin_b, conv_w, conv_b, fc_w, fc_b):
    out, _ = _run(
        {
            "x": x,
            "edge_w": edge_w,
            "lin_w": lin_w,
            "lin_b": lin_b,
            "conv_w": conv_w,
            "conv_b": conv_b,
            "fc_w": fc_w,
            "fc_b": fc_b,
        }
    )
    return out

